# revision 6
# baseline (speedup 1.0000x reference)
"""GQA attention prefill (B=4, S=2048, D=4096, 32 q-heads / 8 kv-heads, rotary,
causal) on 8 TRN2 NeuronCores.

Sharding: token-parallel ("zigzag" sequence split) — core c handles batch
c//2 and two 512-token chunks of its sequence: chunks {0,3} for even cores,
{1,2} for odd cores (balances the causal triangle). Each core computes full
Q projection for its tokens, K/V for the whole prefix, attention for all 32
heads over its tokens, and the output projection for its tokens. No
inter-core communication: outputs are disjoint token slices, gathered on
host.

Speed plan: fp8e4 DoubleRow matmuls process 2x128 contraction rows in 256
PE cycles (~4x bf16 throughput).  The attention softmax here is PEAKED
(diagonal q.k scores reach ~11 sigma), so plain fp8 projections are too
lossy.  Instead Q/K/V projections run a 3-pass fp8 residual decomposition

    W.x ~= Whi.xhi + Whi.xlo + Wlo.xhi      (each term exact in f32 PSUM)

with Whi = e4m3(32W), Wlo = e4m3(32W - Whi) (subnormal-range residual),
xhi = e4m3(x), xlo = e4m3(x - xhi).  Leftover error ~1e-3 — better than a
bf16 matmul at 48/32 = 1.5x the instructions but 4x the rate (~23% faster).
The x32 psum scale folds into the rotary cos/sin tables and evict affines.

Attention itself runs fp16 (q/k/v/pt stored fp16; scores, AV and the
ones-vector denominator matmuls at full bf16-class rate; exp(s - ln8)
keeps pt under fp16 max for |s| <= 13).  The O projection runs 1-pass
fp8 DoubleRow (errors are relative to the small attention outputs) except
token columns 0-127 of slot 0 (the causal-start tokens, where outputs are
O(1) and sharp softmax gives no averaging) which stay bf16: slot-0 evicts
write bf16 cols 0-127 + a x16 fp8 copy; slot-1 evicts write fp8 only.

Device layout conventions (as baseline):
  - activations for QK^T kept transposed [head_dim, tokens]; rotary pairs
    de-interleaved host-side (even dims -> partitions 0-63, odd -> 64-127).
  - scores-transposed softmax over the key (partition) axis; denominator
    via ones-vector matmul; no max-subtraction.
  - DMA routing: large batched loads on nc.sync (HWDGE); high-count
    weight/output streams on nc.gpsimd (SWDGE on the Pool engine).
"""

import numpy as np
import ml_dtypes

import concourse.bacc as bacc
import concourse.bass as bass
import concourse.tile as tile
from concourse import library_config, mybir
from concourse.bass_utils import run_bass_kernel_spmd

F32 = mybir.dt.float32
F32R = mybir.dt.float32r
BF16 = mybir.dt.bfloat16
FP16 = mybir.dt.float16
FP8 = mybir.dt.float8e4
DRM = mybir.MatmulPerfMode.DoubleRow
EXP = mybir.ActivationFunctionType.Exp
COPY = mybir.ActivationFunctionType.Copy
ADD = mybir.AluOpType.add
MULT = mybir.AluOpType.mult

B, S, D = 4, 2048, 4096
QH, KVH, HEAD = 32, 8, 128
P = 128
CH = 512                # token chunk (= query tile)
NCH = S // CH           # 4 chunks per sequence
ND = D // P             # 32 d-tiles
NDP = ND // 2           # 16 dt-pairs (DoubleRow)
NCORES = 8
NKB = (8, 16)           # key-blocks per query slot (padded, uniform)
SCALE = 1.0 / np.sqrt(HEAD)
EXPB = -np.log(8.0)     # exp(s - ln8): fp16-safe for |s| <= 13
WSC = 32.0              # fp8 weight pre-scale (host)
OSC = 16.0              # fp8 attention-output pre-scale (device)
BF = ml_dtypes.bfloat16
F16 = np.float16
E4 = ml_dtypes.float8_e4m3

_CACHE = {}


def _build():
    nc = bacc.Bacc("TRN2", target_bir_lowering=False, debug=False, num_devices=NCORES)

    # ---- per-core external inputs ----
    own_hi = nc.dram_tensor("own_hi", [2, P, ND, CH], FP8, kind="ExternalInput")
    own_lo = nc.dram_tensor("own_lo", [2, P, ND, CH], FP8, kind="ExternalInput")
    pref_hi = nc.dram_tensor("pref_hi", [NCH, P, ND, CH], FP8, kind="ExternalInput")
    pref_lo = nc.dram_tensor("pref_lo", [NCH, P, ND, CH], FP8, kind="ExternalInput")
    qw_hi = nc.dram_tensor("qw_hi", [QH, P, ND, P], FP8, kind="ExternalInput")
    qw_lo = nc.dram_tensor("qw_lo", [QH, P, ND, P], FP8, kind="ExternalInput")
    kw_hi = nc.dram_tensor("kw_hi", [KVH, P, ND, P], FP8, kind="ExternalInput")
    kw_lo = nc.dram_tensor("kw_lo", [KVH, P, ND, P], FP8, kind="ExternalInput")
    vw_hi = nc.dram_tensor("vw_hi", [2, ND, P, 512], FP8, kind="ExternalInput")
    vw_lo = nc.dram_tensor("vw_lo", [2, ND, P, 512], FP8, kind="ExternalInput")
    owT_lo = nc.dram_tensor("owT_lo", [8, ND, P, 512], FP8, kind="ExternalInput")
    owT_8 = nc.dram_tensor("owT_8", [8, ND, P, 512], FP8, kind="ExternalInput")
    cos_own = nc.dram_tensor("cos_own", [64, 2, CH], F32, kind="ExternalInput")
    sin_own = nc.dram_tensor("sin_own", [64, 2, CH], F32, kind="ExternalInput")
    cos_all = nc.dram_tensor("cos_all", [64, S], F32, kind="ExternalInput")
    sin_all = nc.dram_tensor("sin_all", [64, S], F32, kind="ExternalInput")
    qbT = nc.dram_tensor("qbT", [P, QH], F32, kind="ExternalInput")
    kbT = nc.dram_tensor("kbT", [P, KVH], F32, kind="ExternalInput")
    vb = nc.dram_tensor("vb", [KVH * HEAD], F32, kind="ExternalInput")
    ob = nc.dram_tensor("ob", [D], F32, kind="ExternalInput")
    masks = nc.dram_tensor("masks", [2, 8, P, CH], FP16, kind="ExternalInput")
    ones = nc.dram_tensor("ones", [P], FP16, kind="ExternalInput")

    # ---- internal DRAM ----
    qT_i = nc.dram_tensor("qT_i", [2, QH, P, CH], FP16)
    kT_i = nc.dram_tensor("kT_i", [KVH, P, S], FP16)
    v_i = nc.dram_tensor("v_i", [2, 16, P, 512], FP16)   # [hs, kb, kj, j*128+hd]
    oT_lo_i = nc.dram_tensor("oT_lo_i", [QH, P, P], FP8)  # slot0 o x16 residual
    oT8_i = nc.dram_tensor("oT8_i", [2, QH, P, CH], FP8)  # x OSC

    out = nc.dram_tensor("out", [8, P, D], F32, kind="ExternalOutput")

    with tile.TileContext(nc) as tc:
        nc.gpsimd.load_library(library_config.lib)
        with (
            tc.tile_pool(name="const", bufs=1) as const,
            tc.tile_pool(name="ev", bufs=6) as evpool,
            tc.tile_pool(name="rt", bufs=4) as rtpool,
            tc.tile_pool(name="ps", bufs=8, space="PSUM") as pspool,
        ):
            # biases pre-scaled x WSC host-side (all proj psums are x WSC)
            kbT_e = const.tile([64, KVH], F32, tag="kbte")
            kbT_o = const.tile([64, KVH], F32, tag="kbto")
            nc.sync.dma_start(out=kbT_e[:], in_=kbT[0:64, :])
            nc.sync.dma_start(out=kbT_o[:], in_=kbT[64:P, :])
            ones_col = const.tile([P, 1], FP16, tag="oc")
            nc.sync.dma_start(out=ones_col[:], in_=ones.ap()[:, None])
            expb_sb = const.tile([P, 1], F32, tag="expb")
            nc.gpsimd.memset(expb_sb[:], EXPB)

            def rotary_evict(ps, dst, cos_ap, sin_ap, be, bo):
                """dst[0:64]=(pe+be)*cos-(po+bo)*sin; dst[64:128]=(pe+be)*sin+(po+bo)*cos

                cos/sin tables are pre-divided by WSC; be/bo pre-multiplied."""
                pe, po = ps[0:64, :], ps[64:128, :]
                t1 = rtpool.tile([64, CH], F32, tag="rt", name="t1")
                t2 = rtpool.tile([64, CH], F32, tag="rt", name="t2")
                nc.vector.scalar_tensor_tensor(t1[:], pe, be, cos_ap, ADD, MULT)
                nc.vector.scalar_tensor_tensor(t2[:], po, bo, sin_ap, ADD, MULT)
                nc.vector.tensor_sub(dst[0:64, :], t1[:], t2[:])
                t3 = rtpool.tile([64, CH], F32, tag="rt", name="t3")
                t4 = rtpool.tile([64, CH], F32, tag="rt", name="t4")
                nc.vector.scalar_tensor_tensor(t3[:], pe, be, sin_ap, ADD, MULT)
                nc.vector.scalar_tensor_tensor(t4[:], po, bo, cos_ap, ADD, MULT)
                nc.vector.tensor_add(dst[64:128, :], t3[:], t4[:])

            def proj3(ps, whi, wlo, shi, slo, yield_every=0):
                """48 DR matmuls: Whi.xhi + Whi.xlo + Wlo.xhi into ps."""
                n = 0
                for wp, sp_ in ((whi, None), (wlo, shi)):
                    for dp in range(NDP):
                        sl2 = slice(2 * dp, 2 * dp + 2)
                        if sp_ is None:
                            for mv in (shi, slo):
                                nc.tensor.matmul(
                                    ps[:], lhsT=whi[:, sl2, :], rhs=mv[:, sl2, :],
                                    start=(dp == 0 and mv is shi), stop=False,
                                    perf_mode=DRM,
                                )
                                n += 1
                                if yield_every and n % yield_every == 0:
                                    yield
                        else:
                            nc.tensor.matmul(
                                ps[:], lhsT=wlo[:, sl2, :], rhs=shi[:, sl2, :],
                                start=False, stop=(dp == NDP - 1), perf_mode=DRM,
                            )
                            n += 1
                            if yield_every and n % yield_every == 0:
                                yield

            # ====== shared pools for P1 + P0 ======
            w_cm = tc.tile_pool(name="w", bufs=2)
            wpool = w_cm.__enter__()
            p01_cm = tc.tile_pool(name="strip", bufs=5)
            strip_pool = p01_cm.__enter__()

            # ============ P1: K/V projection over full prefix ============
            with tc.tile_pool(name="p1c", bufs=1) as p1c:
                cos_all_sb = p1c.tile([64, S], F32, tag="cosa")
                sin_all_sb = p1c.tile([64, S], F32, tag="sina")
                nc.sync.dma_start(out=cos_all_sb[:], in_=cos_all[:])
                nc.sync.dma_start(out=sin_all_sb[:], in_=sin_all[:])
                vb_sb = p1c.tile([P, KVH * HEAD], F32, tag="vb")
                nc.sync.dma_start(
                    out=vb_sb[:], in_=vb.ap()[None, :].partition_broadcast(P)
                )
                with tc.tile_pool(name="wb", bufs=6) as wbpool:
                    for pr in range(2):
                        sts = []
                        for i in range(2):
                            c = 2 * pr + i
                            shi = strip_pool.tile(
                                [P, ND, CH], FP8, tag="strip", name=f"shi{c}"
                            )
                            nc.sync.dma_start(out=shi[:], in_=pref_hi[c])
                            slo = strip_pool.tile(
                                [P, ND, CH], FP8, tag="strip", name=f"slo{c}"
                            )
                            nc.sync.dma_start(out=slo[:], in_=pref_lo[c])
                            sts.append((shi, slo))
                        # ---- K-pass (weight-stationary, 3-pass DR) ----
                        for kv in range(KVH):
                            whi = wpool.tile([P, ND, P], FP8, tag="w", name=f"kwh{kv}")
                            nc.gpsimd.dma_start(out=whi[:], in_=kw_hi[kv])
                            wlo = wpool.tile([P, ND, P], FP8, tag="wl", name=f"kwl{kv}")
                            nc.gpsimd.dma_start(out=wlo[:], in_=kw_lo[kv])
                            for ts in range(2):
                                tg = 2 * pr + ts
                                ps = pspool.tile([P, CH], F32, tag="ps", name="ps_k")
                                for _ in proj3(ps, whi, wlo, sts[ts][0], sts[ts][1]):
                                    pass
                                krot = evpool.tile([P, CH], FP16, tag="ev", name="krot")
                                rotary_evict(
                                    ps, krot,
                                    cos_all_sb[:, tg * CH : (tg + 1) * CH],
                                    sin_all_sb[:, tg * CH : (tg + 1) * CH],
                                    kbT_e[:, kv : kv + 1], kbT_o[:, kv : kv + 1],
                                )
                                nc.sync.dma_start(
                                    out=kT_i[kv, :, tg * CH : (tg + 1) * CH], in_=krot[:]
                                )
                        # ---- V-pass (input-stationary, 3-pass DR) ----
                        for hs in range(2):
                            psv = [
                                pspool.tile([P, 512], F32, tag="ps", name=f"psv{i}")
                                for i in range(8)
                            ]
                            for dp in range(NDP):
                                sl2 = slice(2 * dp, 2 * dp + 2)
                                vwh = wbpool.tile([P, 2, 512], FP8, tag="wbh", name="vwh")
                                nc.gpsimd.dma_start(
                                    out=vwh[:],
                                    in_=vw_hi[hs, sl2].rearrange("d p j -> p d j"),
                                )
                                vwl = wbpool.tile([P, 2, 512], FP8, tag="wbl", name="vwl")
                                nc.gpsimd.dma_start(
                                    out=vwl[:],
                                    in_=vw_lo[hs, sl2].rearrange("d p j -> p d j"),
                                )
                                for ts in range(2):
                                    shi, slo = sts[ts]
                                    for tt in range(4):
                                        tsl = slice(tt * P, (tt + 1) * P)
                                        bank = psv[ts * 4 + tt][:]
                                        nc.tensor.matmul(
                                            bank, lhsT=shi[:, sl2, tsl], rhs=vwh[:],
                                            start=(dp == 0), stop=False, perf_mode=DRM,
                                        )
                                        nc.tensor.matmul(
                                            bank, lhsT=slo[:, sl2, tsl], rhs=vwh[:],
                                            start=False, stop=False, perf_mode=DRM,
                                        )
                                        nc.tensor.matmul(
                                            bank, lhsT=shi[:, sl2, tsl], rhs=vwl[:],
                                            start=False, stop=(dp == NDP - 1),
                                            perf_mode=DRM,
                                        )
                            for ts in range(2):
                                for tt in range(4):
                                    kb = (2 * pr + ts) * 4 + tt
                                    vsb = evpool.tile([P, 512], FP16, tag="evb", name="vsb")
                                    nc.vector.scalar_tensor_tensor(
                                        vsb[:], psv[ts * 4 + tt][:], 1.0 / WSC,
                                        vb_sb[:, hs * 512 : (hs + 1) * 512], MULT, ADD,
                                    )
                                    nc.sync.dma_start(out=v_i[hs, kb], in_=vsb[:])

            p01_cm.__exit__(None, None, None)
            kv_cm = tc.tile_pool(name="kvS", bufs=4)
            kvpool = kv_cm.__enter__()
            qt_cm = tc.tile_pool(name="qtS", bufs=3)
            qtpool = qt_cm.__enter__()
            pt_cm = tc.tile_pool(name="ptS", bufs=6)
            ptpool = pt_cm.__enter__()
            r_cm = tc.tile_pool(name="rS", bufs=2)
            rpool = r_cm.__enter__()
            p0s_cm = tc.tile_pool(name="p0strip", bufs=4)
            p0strip_pool = p0s_cm.__enter__()

            # ============ P0: Q projection + rotary -> qT_i ============
            p0c_cm = tc.tile_pool(name="p0c", bufs=1)
            p0c = p0c_cm.__enter__()
            cos_own_sb = p0c.tile([64, 2, CH], F32, tag="coso")
            sin_own_sb = p0c.tile([64, 2, CH], F32, tag="sino")
            nc.sync.dma_start(out=cos_own_sb[:], in_=cos_own[:])
            nc.sync.dma_start(out=sin_own_sb[:], in_=sin_own[:])
            qbT_e = p0c.tile([64, QH], F32, tag="qbte")
            qbT_o = p0c.tile([64, QH], F32, tag="qbto")
            nc.sync.dma_start(out=qbT_e[:], in_=qbT[0:64, :])
            nc.sync.dma_start(out=qbT_o[:], in_=qbT[64:P, :])

            def p0_heads(sl, yield_every):
                """Emit Q-proj for one slot (3-pass DR); yields as feeder."""
                shi = p0strip_pool.tile([P, ND, CH], FP8, tag="p0strip", name=f"oh{sl}")
                nc.sync.dma_start(out=shi[:], in_=own_hi[sl])
                slo = p0strip_pool.tile([P, ND, CH], FP8, tag="p0strip", name=f"ol{sl}")
                nc.sync.dma_start(out=slo[:], in_=own_lo[sl])
                for h in range(QH):
                    whi = wpool.tile([P, ND, P], FP8, tag="w", name=f"qwh{sl}_{h}")
                    nc.gpsimd.dma_start(out=whi[:], in_=qw_hi[h])
                    wlo = wpool.tile([P, ND, P], FP8, tag="wl", name=f"qwl{sl}_{h}")
                    nc.gpsimd.dma_start(out=wlo[:], in_=qw_lo[h])
                    ps = pspool.tile([P, CH], F32, tag="ps", name="ps_q")
                    yield from proj3(ps, whi, wlo, shi, slo, yield_every=yield_every)
                    qrot = evpool.tile([P, CH], FP16, tag="ev", name="qrot")
                    rotary_evict(
                        ps, qrot,
                        cos_own_sb[:, sl, :], sin_own_sb[:, sl, :],
                        qbT_e[:, h : h + 1], qbT_o[:, h : h + 1],
                    )
                    nc.sync.dma_start(out=qT_i[sl, h], in_=qrot[:])

            for _ in p0_heads(0, 0):
                pass

            def p4_half(hf, otr8, otrb, wb4pool, ob_sb):
                """O proj for token-slot half hf: 1-pass fp8 DR; slot-0 token
                cols 0-127 (tsub 0) stay bf16.  Yields once per matmul."""
                for hq in range(0, QH, 8):
                    nc.sync.dma_start(
                        out=otr8[:, hq : hq + 8, :],
                        in_=oT8_i[hf, hq : hq + 8].rearrange("h p t -> p h t"),
                    )
                    if hf == 0:
                        nc.sync.dma_start(
                            out=otrb[:, hq : hq + 8, :],
                            in_=oT_lo_i[hq : hq + 8].rearrange("h p t -> p h t"),
                        )
                for e in range(8):
                    ps4 = [
                        pspool.tile([P, 512], F32, tag="ps", name=f"ps4_{i}")
                        for i in range(4)
                    ]
                    for f4 in range(ND // 4):
                        ow8 = wb4pool.tile([P, 4, 512], FP8, tag="wb4", name="ow8")
                        nc.gpsimd.dma_start(
                            out=ow8[:],
                            in_=owT_8[e, 4 * f4 : 4 * f4 + 4].rearrange("d p j -> p d j"),
                        )
                        if hf == 0:
                            owl = wb4pool.tile([P, 4, 512], FP8, tag="wbb", name="owl")
                            nc.gpsimd.dma_start(
                                out=owl[:],
                                in_=owT_lo[e, 4 * f4 : 4 * f4 + 4].rearrange(
                                    "d p j -> p d j"
                                ),
                            )
                        for dfp in range(2):
                            ft = 4 * f4 + 2 * dfp
                            wsl = slice(2 * dfp, 2 * dfp + 2)
                            if hf == 0:
                                nc.tensor.matmul(
                                    ps4[0][:], lhsT=otr8[:, ft : ft + 2, 0:P],
                                    rhs=ow8[:, wsl, :],
                                    start=(ft == 0), stop=False, perf_mode=DRM,
                                )
                                yield
                                nc.tensor.matmul(
                                    ps4[0][:], lhsT=otrb[:, ft : ft + 2, :],
                                    rhs=ow8[:, wsl, :],
                                    start=False, stop=False, perf_mode=DRM,
                                )
                                yield
                                nc.tensor.matmul(
                                    ps4[0][:], lhsT=otr8[:, ft : ft + 2, 0:P],
                                    rhs=owl[:, wsl, :],
                                    start=False, stop=(ft == ND - 2),
                                    perf_mode=DRM,
                                )
                                yield
                            else:
                                nc.tensor.matmul(
                                    ps4[0][:], lhsT=otr8[:, ft : ft + 2, 0:P],
                                    rhs=ow8[:, wsl, :],
                                    start=(ft == 0), stop=(ft == ND - 2),
                                    perf_mode=DRM,
                                )
                                yield
                            for tsub in range(1, 4):
                                nc.tensor.matmul(
                                    ps4[tsub][:],
                                    lhsT=otr8[:, ft : ft + 2,
                                             tsub * P : (tsub + 1) * P],
                                    rhs=ow8[:, wsl, :],
                                    start=(ft == 0), stop=(ft == ND - 2),
                                    perf_mode=DRM,
                                )
                                yield
                    for tsub in range(4):
                        osb = evpool.tile([P, 512], F32, tag="ev4", name="osb4")
                        obias = ob_sb[:, e * 512 : (e + 1) * 512]
                        nc.vector.scalar_tensor_tensor(
                            osb[:], ps4[tsub][:], 1.0 / (OSC * WSC),
                            obias, MULT, ADD,
                        )
                        nc.sync.dma_start(
                            out=out[hf * 4 + tsub, :, e * 512 : (e + 1) * 512],
                            in_=osb[:],
                        )

            def attn_slot(sl, feeder):
                n_kb = NKB[sl]
                with (
                    tc.tile_pool(name=f"mask{sl}", bufs=1) as mpool,
                    tc.tile_pool(name=f"v4{sl}", bufs=1) as v4pool,
                ):
                    msk = mpool.tile([P, 8, CH], FP16, tag="mask", name="msk")
                    nc.sync.dma_start(
                        out=msk[:], in_=masks[sl].rearrange("m k q -> k m q")
                    )
                    for hs in range(2):
                        v4 = v4pool.tile([P, n_kb, 512], FP16, tag="v4", name="v4")
                        nc.sync.dma_start(
                            out=v4[:, 0:n_kb, :],
                            in_=v_i[hs, 0:n_kb].rearrange("b p j -> p b j"),
                        )
                        for j in range(4):
                            kv = 4 * hs + j
                            kt = kvpool.tile([P, n_kb * P], FP16, tag="kt", name="kt")
                            nc.sync.dma_start(
                                out=kt[:, 0 : n_kb * P], in_=kT_i[kv, :, 0 : n_kb * P]
                            )
                            qt4 = qtpool.tile([P, 4, CH], FP16, tag="qt", name="qt4")
                            nc.sync.dma_start(
                                out=qt4[:],
                                in_=qT_i[sl, kv :: KVH].rearrange("g p t -> p g t"),
                            )
                            for g in range(4):
                                h = kv + KVH * g
                                oT_ps = pspool.tile([P, CH], F32, tag="ps", name="oT_ps")
                                sums_ps = pspool.tile([P, CH], F32, tag="ps", name="sums_ps")
                                for kb in range(n_kb):
                                    st_ps = pspool.tile([P, CH], F32, tag="ps", name="st_ps")
                                    nc.tensor.matmul(
                                        st_ps[:],
                                        lhsT=kt[:, kb * P : (kb + 1) * P],
                                        rhs=qt4[:, g, :], start=True, stop=True,
                                    )
                                    pt = ptpool.tile([P, CH], FP16, tag="pt", name="pt")
                                    nc.scalar.activation(
                                        pt[:], st_ps[:], EXP,
                                        scale=SCALE, bias=expb_sb[:],
                                    )
                                    if sl == 0 or kb >= 8:
                                        mi = kb if sl == 0 else kb - 8
                                        nc.vector.tensor_mul(pt[:], pt[:], msk[:, mi, :])
                                    if feeder is not None:
                                        next(feeder, None)
                                    if kb % 2 == 0:
                                        pt_prev = pt
                                    else:
                                        pp = ptpool.tile([P, CH], FP16, tag="ptp", name="pp")
                                        nc.vector.tensor_add(pp[:], pt_prev[:], pt[:])
                                        if kb % 4 == 1:
                                            pp_prev = pp
                                        else:
                                            pq = ptpool.tile([P, CH], FP16, tag="ptq", name="pq")
                                            nc.vector.tensor_add(pq[:], pp_prev[:], pp[:])
                                            nc.tensor.matmul(
                                                sums_ps[0:1, :], lhsT=ones_col[:], rhs=pq[:],
                                                start=(kb == 3), stop=(kb == n_kb - 1),
                                            )
                                    if feeder is not None:
                                        next(feeder, None)
                                    nc.tensor.matmul(
                                        oT_ps[:],
                                        lhsT=v4[:, kb, j * P : (j + 1) * P],
                                        rhs=pt[:],
                                        start=(kb == 0), stop=(kb == n_kb - 1),
                                    )
                                rsb = rpool.tile([1, CH], F32R, tag="r", name="rsb")
                                with nc.allow_low_precision(reason="f32r softmax denom"):
                                    nc.vector.reciprocal(rsb[:], sums_ps[0:1, :])
                                rb_bc = ptpool.tile([P, CH], F32R, tag="ptr", name="rb_bc")
                                nc.gpsimd.partition_broadcast(rb_bc[:], rsb[:])
                                osb8 = evpool.tile([P, CH], FP8, tag="ev8", name="osb8")
                                nc.vector.scalar_tensor_tensor(
                                    osb8[:], oT_ps[:], OSC, rb_bc[:], MULT, MULT
                                )
                                nc.sync.dma_start(out=oT8_i[sl, h], in_=osb8[:])
                                if sl == 0:
                                    o16 = evpool.tile([P, P], F32, tag="evb", name="o16")
                                    nc.vector.scalar_tensor_tensor(
                                        o16[:], oT_ps[:, 0:P], OSC, rb_bc[:, 0:P],
                                        MULT, MULT,
                                    )
                                    olo8 = evpool.tile([P, P], FP8, tag="evl", name="olo8")
                                    nc.vector.tensor_sub(olo8[:], o16[:], osb8[:, 0:P])
                                    nc.sync.dma_start(out=oT_lo_i[h], in_=olo8[:])
                    if feeder is not None:
                        for _ in feeder:
                            pass

            # ==== P3 slot 0 woven with P0 slot 1 ====
            attn_slot(0, p0_heads(1, 3))
            p0c_cm.__exit__(None, None, None)
            p0s_cm.__exit__(None, None, None)

            # ==== P3 slot 1 woven with P4 half 0; then P4 half 1 ====
            with tc.tile_pool(name="obp", bufs=1) as obp:
                ob_sb = obp.tile([P, D], F32, tag="ob")
                nc.sync.dma_start(
                    out=ob_sb[:], in_=ob.ap()[None, :].partition_broadcast(P)
                )
                with (
                    tc.tile_pool(name="p4a", bufs=1) as p4a,
                    tc.tile_pool(name="wb4a", bufs=4) as wb4a,
                ):
                    otr8a = p4a.tile([P, QH, CH], FP8, tag="ot8a")
                    otrba = p4a.tile([P, QH, P], FP8, tag="otba")
                    attn_slot(1, p4_half(0, otr8a, otrba, wb4a, ob_sb))
                with (
                    tc.tile_pool(name="p4b", bufs=1) as p4b,
                    tc.tile_pool(name="wb4b", bufs=6) as wb4b,
                ):
                    otr8b = p4b.tile([P, QH, CH], FP8, tag="ot8b")
                    for _ in p4_half(1, otr8b, None, wb4b, ob_sb):
                        pass
            r_cm.__exit__(None, None, None)
            pt_cm.__exit__(None, None, None)
            qt_cm.__exit__(None, None, None)
            kv_cm.__exit__(None, None, None)
            w_cm.__exit__(None, None, None)

    nc.compile()
    return nc


def _get_nc():
    if "nc" not in _CACHE:
        _CACHE["nc"] = _build()
    return _CACHE["nc"]


_PERM = np.concatenate([np.arange(0, P, 2), np.arange(1, P, 2)])


def _hi_lo(a):
    """fp8 residual split: hi = e4m3(a); lo = e4m3(a - hi)."""
    c = np.ascontiguousarray
    hi = a.astype(np.float32).astype(E4)
    lo = (a.astype(np.float32) - hi.astype(np.float32)).astype(E4)
    return c(hi), c(lo)


def _prep_shared(qw_w, qw_b, kw_w, kw_b, vw_w, vw_b, ow_w, ow_b, fc, fs):
    f32 = np.float32
    c = np.ascontiguousarray
    # [h, dp, dt, fp] = w[h*128 + perm[fp], dt*128 + dp]
    qq = qw_w.reshape(QH, P, D)[:, _PERM, :]                      # [h, fp, d]
    qwT = qq.reshape(QH, P, ND, P).transpose(0, 3, 2, 1)
    kk = kw_w.reshape(KVH, P, D)[:, _PERM, :]
    kwT = kk.reshape(KVH, P, ND, P).transpose(0, 3, 2, 1)
    # [hs, dt, dp, j] = vw[hs*512 + j, dt*128 + dp]
    vwT = vw_w.reshape(2, 512, ND, P).transpose(0, 2, 3, 1)
    # [es, ft, fp, j] = ow[es*512 + j, ft*128 + fp]
    owT = ow_w.reshape(8, 512, ND, P).transpose(0, 2, 3, 1)
    ow_hi, ow_lo = _hi_lo(owT * WSC)
    qw_hi, qw_lo = _hi_lo(qwT * WSC)
    kw_hi, kw_lo = _hi_lo(kwT * WSC)
    vw_hi, vw_lo = _hi_lo(vwT * WSC)
    # rotary tables pre-divided by WSC; proj biases pre-multiplied
    cos_all = c(fc.T.astype(f32) / WSC)  # [64, S]
    sin_all = c(fs.T.astype(f32) / WSC)
    qbT = c(qw_b.reshape(QH, P)[:, _PERM].T.astype(f32) * WSC)
    kbT = c(kw_b.reshape(KVH, P)[:, _PERM].T.astype(f32) * WSC)
    return dict(
        qw_hi=qw_hi, qw_lo=qw_lo, kw_hi=kw_hi, kw_lo=kw_lo,
        vw_hi=vw_hi, vw_lo=vw_lo,
        owT_8=ow_hi, owT_lo=ow_lo,
        cos_all=cos_all, sin_all=sin_all, qbT=qbT, kbT=kbT,
        vb=c(vw_b.astype(f32)), ob=c(ow_b.astype(f32)),
    )


def _masks_for(chunks):
    m = np.zeros((2, 8, P, CH), F16)
    kp = np.arange(P)[:, None]
    qi = np.arange(CH)[None, :]
    for sl in range(2):
        q0 = chunks[sl] * CH
        for mi in range(8):
            kb = mi if sl == 0 else mi + 8
            m[sl, mi] = (kb * P + kp <= q0 + qi).astype(F16)
    return m


def _core_chunks(core):
    b, par = core // 2, core % 2
    return b, ((0, 3) if par == 0 else (1, 2))


def _make_in_maps(inputs):
    """inputs: dict with the reference's setup_inputs() keys (numpy)."""
    g = lambda k: np.asarray(inputs[k])
    shared = _prep_shared(
        g("qw_w"), g("qw_b"), g("kw_w"), g("kw_b"), g("vw_w"), g("vw_b"),
        g("ow_w"), g("ow_b"), g("freqs_cos"), g("freqs_sin"),
    )
    input = g("input")
    in_maps = []
    prepped = {}
    for core in range(NCORES):
        b, chunks = _core_chunks(core)
        if b not in prepped:
            x = input[b].astype(np.float32)  # [S, D]
            # [s, dp, dt, t] = x[s*512 + t, dt*128 + dp]
            strips = x.reshape(NCH, CH, ND, P).transpose(0, 3, 2, 1)
            prepped[b] = _hi_lo(strips)
        phi, plo = prepped[b]
        cos_own = np.ascontiguousarray(
            np.stack([shared["cos_all"][:, c * CH : (c + 1) * CH] for c in chunks], 1)
        )
        sin_own = np.ascontiguousarray(
            np.stack([shared["sin_all"][:, c * CH : (c + 1) * CH] for c in chunks], 1)
        )
        m = dict(shared)
        m.update(
            ones=np.ones(P, F16),
            own_hi=np.ascontiguousarray(phi[list(chunks)]),
            own_lo=np.ascontiguousarray(plo[list(chunks)]),
            pref_hi=phi, pref_lo=plo,
            cos_own=cos_own, sin_own=sin_own, masks=_masks_for(chunks),
        )
        in_maps.append(m)
    return in_maps


def kernel(input, freqs_cos, freqs_sin, qw_w, qw_b, kw_w, kw_b, vw_w, vw_b,
           ow_w, ow_b, start_pos):
    in_maps = _make_in_maps(dict(
        input=input, freqs_cos=freqs_cos, freqs_sin=freqs_sin,
        qw_w=qw_w, qw_b=qw_b, kw_w=kw_w, kw_b=kw_b, vw_w=vw_w, vw_b=vw_b,
        ow_w=ow_w, ow_b=ow_b,
    ))
    nc = _get_nc()
    res = run_bass_kernel_spmd(nc, in_maps, list(range(NCORES)))

    out = np.empty((B, S, D), np.float32)
    for core in range(NCORES):
        b, chunks = _core_chunks(core)
        r = res.results[core]["out"].reshape(2, CH, D)
        for sl in range(2):
            c0 = chunks[sl] * CH
            out[b, c0 : c0 + CH, :] = r[sl]
    return out


# revision 7
# speedup vs baseline: 1.0078x; 1.0078x over previous
"""GQA attention prefill (B=4, S=2048, D=4096, 32 q-heads / 8 kv-heads, rotary,
causal) on 8 TRN2 NeuronCores.

Sharding: token-parallel ("zigzag" sequence split) — core c handles batch
c//2 and two 512-token chunks of its sequence: chunks {0,3} for even cores,
{1,2} for odd cores (balances the causal triangle). Each core computes full
Q projection for its tokens, K/V for the whole prefix, attention for all 32
heads over its tokens, and the output projection for its tokens. No
inter-core communication: outputs are disjoint token slices, gathered on
host.

Speed plan: fp8e4 DoubleRow matmuls process 2x128 contraction rows in 256
PE cycles (~4x bf16 throughput).  The attention softmax here is PEAKED
(diagonal q.k scores reach ~11 sigma), so plain fp8 projections are too
lossy.  Instead Q/K/V projections run a 3-pass fp8 residual decomposition

    W.x ~= Whi.xhi + Whi.xlo + Wlo.xhi      (each term exact in f32 PSUM)

with Whi = e4m3(32W), Wlo = e4m3(32W - Whi) (subnormal-range residual),
xhi = e4m3(x), xlo = e4m3(x - xhi).  Leftover error ~1e-3 — better than a
bf16 matmul at 48/32 = 1.5x the instructions but 4x the rate (~23% faster).
The x32 psum scale folds into the rotary cos/sin tables and evict affines.

Attention itself runs fp16 (q/k/v/pt stored fp16; scores, AV and the
ones-vector denominator matmuls at full bf16-class rate; exp(s - ln8)
keeps pt under fp16 max for |s| <= 13).  The O projection runs 1-pass
fp8 DoubleRow (errors are relative to the small attention outputs) except
token columns 0-127 of slot 0 (the causal-start tokens, where outputs are
O(1) and sharp softmax gives no averaging) which run the same 3-pass
residual scheme: slot-0 evicts write a x16 fp8 o plus an fp8 residual of
its first 128 columns; slot-1 evicts write fp8 only.

Scheduling: the PE stream is kept gap-free (the cost model halves PE clock
after any idle gap) by weaving independent DoubleRow matmuls into the
attention dependency windows — Q-slot-1 projection inside slot-0
attention, O-proj half-0 inside slot-1 attention — with two fill points
per key-block placed between the scores matmul and the AV matmul, where
the in-order queue would otherwise stall on the exp -> mask -> pt chain.

Device layout conventions (as baseline):
  - activations for QK^T kept transposed [head_dim, tokens]; rotary pairs
    de-interleaved host-side (even dims -> partitions 0-63, odd -> 64-127).
  - scores-transposed softmax over the key (partition) axis; denominator
    via ones-vector matmul; no max-subtraction.
  - DMA routing: large batched loads on nc.sync (HWDGE); high-count
    weight/output streams on nc.gpsimd (SWDGE on the Pool engine).
"""

import numpy as np
import ml_dtypes

import concourse.bacc as bacc
import concourse.bass as bass
import concourse.tile as tile
from concourse import library_config, mybir
from concourse.bass_utils import run_bass_kernel_spmd

F32 = mybir.dt.float32
F32R = mybir.dt.float32r
BF16 = mybir.dt.bfloat16
FP16 = mybir.dt.float16
FP8 = mybir.dt.float8e4
DRM = mybir.MatmulPerfMode.DoubleRow
EXP = mybir.ActivationFunctionType.Exp
COPY = mybir.ActivationFunctionType.Copy
ADD = mybir.AluOpType.add
MULT = mybir.AluOpType.mult

B, S, D = 4, 2048, 4096
QH, KVH, HEAD = 32, 8, 128
P = 128
CH = 512                # token chunk (= query tile)
NCH = S // CH           # 4 chunks per sequence
ND = D // P             # 32 d-tiles
NDP = ND // 2           # 16 dt-pairs (DoubleRow)
NCORES = 8
NKB = (8, 16)           # key-blocks per query slot (padded, uniform)
SCALE = 1.0 / np.sqrt(HEAD)
EXPB = -np.log(8.0)     # exp(s - ln8): fp16-safe for |s| <= 13
WSC = 32.0              # fp8 weight pre-scale (host)
OSC = 16.0              # fp8 attention-output pre-scale (device)
BF = ml_dtypes.bfloat16
F16 = np.float16
E4 = ml_dtypes.float8_e4m3

_CACHE = {}


def _build():
    nc = bacc.Bacc("TRN2", target_bir_lowering=False, debug=False, num_devices=NCORES)

    # ---- per-core external inputs ----
    own_hi = nc.dram_tensor("own_hi", [2, P, ND, CH], FP8, kind="ExternalInput")
    own_lo = nc.dram_tensor("own_lo", [2, P, ND, CH], FP8, kind="ExternalInput")
    pref_hi = nc.dram_tensor("pref_hi", [NCH, P, ND, CH], FP8, kind="ExternalInput")
    pref_lo = nc.dram_tensor("pref_lo", [NCH, P, ND, CH], FP8, kind="ExternalInput")
    qw_hi = nc.dram_tensor("qw_hi", [QH, P, ND, P], FP8, kind="ExternalInput")
    qw_lo = nc.dram_tensor("qw_lo", [QH, P, ND, P], FP8, kind="ExternalInput")
    kw_hi = nc.dram_tensor("kw_hi", [KVH, P, ND, P], FP8, kind="ExternalInput")
    kw_lo = nc.dram_tensor("kw_lo", [KVH, P, ND, P], FP8, kind="ExternalInput")
    vw_hi = nc.dram_tensor("vw_hi", [2, ND, P, 512], FP8, kind="ExternalInput")
    vw_lo = nc.dram_tensor("vw_lo", [2, ND, P, 512], FP8, kind="ExternalInput")
    owT_lo = nc.dram_tensor("owT_lo", [8, ND, P, 512], FP8, kind="ExternalInput")
    owT_8 = nc.dram_tensor("owT_8", [8, ND, P, 512], FP8, kind="ExternalInput")
    cos_own = nc.dram_tensor("cos_own", [64, 2, CH], F32, kind="ExternalInput")
    sin_own = nc.dram_tensor("sin_own", [64, 2, CH], F32, kind="ExternalInput")
    cos_all = nc.dram_tensor("cos_all", [64, S], F32, kind="ExternalInput")
    sin_all = nc.dram_tensor("sin_all", [64, S], F32, kind="ExternalInput")
    qbT = nc.dram_tensor("qbT", [P, QH], F32, kind="ExternalInput")
    kbT = nc.dram_tensor("kbT", [P, KVH], F32, kind="ExternalInput")
    vb = nc.dram_tensor("vb", [KVH * HEAD], F32, kind="ExternalInput")
    ob = nc.dram_tensor("ob", [D], F32, kind="ExternalInput")
    masks = nc.dram_tensor("masks", [2, 8, P, CH], FP16, kind="ExternalInput")
    ones = nc.dram_tensor("ones", [P], FP16, kind="ExternalInput")

    # ---- internal DRAM ----
    qT_i = nc.dram_tensor("qT_i", [2, QH, P, CH], FP16)
    kT_i = nc.dram_tensor("kT_i", [KVH, P, S], FP16)
    v_i = nc.dram_tensor("v_i", [2, 16, P, 512], FP16)   # [hs, kb, kj, j*128+hd]
    oT_lo_i = nc.dram_tensor("oT_lo_i", [QH, P, P], FP8)  # slot0 o x16 residual
    oT8_i = nc.dram_tensor("oT8_i", [2, QH, P, CH], FP8)  # x OSC

    out = nc.dram_tensor("out", [8, P, D], F32, kind="ExternalOutput")

    with tile.TileContext(nc) as tc:
        nc.gpsimd.load_library(library_config.lib)
        with (
            tc.tile_pool(name="const", bufs=1) as const,
            tc.tile_pool(name="ev", bufs=6) as evpool,
            tc.tile_pool(name="rt", bufs=4) as rtpool,
            tc.tile_pool(name="ps", bufs=8, space="PSUM") as pspool,
        ):
            # biases pre-scaled x WSC host-side (all proj psums are x WSC)
            kbT_e = const.tile([64, KVH], F32, tag="kbte")
            kbT_o = const.tile([64, KVH], F32, tag="kbto")
            nc.sync.dma_start(out=kbT_e[:], in_=kbT[0:64, :])
            nc.sync.dma_start(out=kbT_o[:], in_=kbT[64:P, :])
            ones_col = const.tile([P, 1], FP16, tag="oc")
            nc.sync.dma_start(out=ones_col[:], in_=ones.ap()[:, None])
            expb_sb = const.tile([P, 1], F32, tag="expb")
            nc.gpsimd.memset(expb_sb[:], EXPB)

            def rotary_evict(ps, dst, cos_ap, sin_ap, be, bo):
                """dst[0:64]=(pe+be)*cos-(po+bo)*sin; dst[64:128]=(pe+be)*sin+(po+bo)*cos

                cos/sin tables are pre-divided by WSC; be/bo pre-multiplied."""
                pe, po = ps[0:64, :], ps[64:128, :]
                t1 = rtpool.tile([64, CH], F32, tag="rt", name="t1")
                t2 = rtpool.tile([64, CH], F32, tag="rt", name="t2")
                nc.vector.scalar_tensor_tensor(t1[:], pe, be, cos_ap, ADD, MULT)
                nc.vector.scalar_tensor_tensor(t2[:], po, bo, sin_ap, ADD, MULT)
                nc.vector.tensor_sub(dst[0:64, :], t1[:], t2[:])
                t3 = rtpool.tile([64, CH], F32, tag="rt", name="t3")
                t4 = rtpool.tile([64, CH], F32, tag="rt", name="t4")
                nc.vector.scalar_tensor_tensor(t3[:], pe, be, sin_ap, ADD, MULT)
                nc.vector.scalar_tensor_tensor(t4[:], po, bo, cos_ap, ADD, MULT)
                nc.vector.tensor_add(dst[64:128, :], t3[:], t4[:])

            def proj3(ps, whi, wlo, shi, slo, yield_every=0):
                """48 DR matmuls: Whi.xhi + Whi.xlo + Wlo.xhi into ps."""
                n = 0
                for wp, sp_ in ((whi, None), (wlo, shi)):
                    for dp in range(NDP):
                        sl2 = slice(2 * dp, 2 * dp + 2)
                        if sp_ is None:
                            for mv in (shi, slo):
                                nc.tensor.matmul(
                                    ps[:], lhsT=whi[:, sl2, :], rhs=mv[:, sl2, :],
                                    start=(dp == 0 and mv is shi), stop=False,
                                    perf_mode=DRM,
                                )
                                n += 1
                                if yield_every and n % yield_every == 0:
                                    yield
                        else:
                            nc.tensor.matmul(
                                ps[:], lhsT=wlo[:, sl2, :], rhs=shi[:, sl2, :],
                                start=False, stop=(dp == NDP - 1), perf_mode=DRM,
                            )
                            n += 1
                            if yield_every and n % yield_every == 0:
                                yield

            # ====== shared pools for P1 + P0 ======
            w_cm = tc.tile_pool(name="w", bufs=2)
            wpool = w_cm.__enter__()
            p01_cm = tc.tile_pool(name="strip", bufs=5)
            strip_pool = p01_cm.__enter__()

            # ============ P1: K/V projection over full prefix ============
            with tc.tile_pool(name="p1c", bufs=1) as p1c:
                cos_all_sb = p1c.tile([64, S], F32, tag="cosa")
                sin_all_sb = p1c.tile([64, S], F32, tag="sina")
                nc.sync.dma_start(out=cos_all_sb[:], in_=cos_all[:])
                nc.sync.dma_start(out=sin_all_sb[:], in_=sin_all[:])
                vb_sb = p1c.tile([P, KVH * HEAD], F32, tag="vb")
                nc.sync.dma_start(
                    out=vb_sb[:], in_=vb.ap()[None, :].partition_broadcast(P)
                )
                with tc.tile_pool(name="wb", bufs=6) as wbpool:
                    for pr in range(2):
                        sts = []
                        for i in range(2):
                            c = 2 * pr + i
                            shi = strip_pool.tile(
                                [P, ND, CH], FP8, tag="strip", name=f"shi{c}"
                            )
                            nc.sync.dma_start(out=shi[:], in_=pref_hi[c])
                            slo = strip_pool.tile(
                                [P, ND, CH], FP8, tag="strip", name=f"slo{c}"
                            )
                            nc.sync.dma_start(out=slo[:], in_=pref_lo[c])
                            sts.append((shi, slo))
                        # ---- K-pass (weight-stationary, 3-pass DR) ----
                        for kv in range(KVH):
                            whi = wpool.tile([P, ND, P], FP8, tag="w", name=f"kwh{kv}")
                            nc.gpsimd.dma_start(out=whi[:], in_=kw_hi[kv])
                            wlo = wpool.tile([P, ND, P], FP8, tag="wl", name=f"kwl{kv}")
                            nc.gpsimd.dma_start(out=wlo[:], in_=kw_lo[kv])
                            for ts in range(2):
                                tg = 2 * pr + ts
                                ps = pspool.tile([P, CH], F32, tag="ps", name="ps_k")
                                for _ in proj3(ps, whi, wlo, sts[ts][0], sts[ts][1]):
                                    pass
                                krot = evpool.tile([P, CH], FP16, tag="ev", name="krot")
                                rotary_evict(
                                    ps, krot,
                                    cos_all_sb[:, tg * CH : (tg + 1) * CH],
                                    sin_all_sb[:, tg * CH : (tg + 1) * CH],
                                    kbT_e[:, kv : kv + 1], kbT_o[:, kv : kv + 1],
                                )
                                nc.sync.dma_start(
                                    out=kT_i[kv, :, tg * CH : (tg + 1) * CH], in_=krot[:]
                                )
                        # ---- V-pass (input-stationary, 3-pass DR) ----
                        for hs in range(2):
                            psv = [
                                pspool.tile([P, 512], F32, tag="ps", name=f"psv{i}")
                                for i in range(8)
                            ]
                            for dp in range(NDP):
                                sl2 = slice(2 * dp, 2 * dp + 2)
                                vwh = wbpool.tile([P, 2, 512], FP8, tag="wbh", name="vwh")
                                nc.gpsimd.dma_start(
                                    out=vwh[:],
                                    in_=vw_hi[hs, sl2].rearrange("d p j -> p d j"),
                                )
                                vwl = wbpool.tile([P, 2, 512], FP8, tag="wbl", name="vwl")
                                nc.gpsimd.dma_start(
                                    out=vwl[:],
                                    in_=vw_lo[hs, sl2].rearrange("d p j -> p d j"),
                                )
                                for ts in range(2):
                                    shi, slo = sts[ts]
                                    for tt in range(4):
                                        tsl = slice(tt * P, (tt + 1) * P)
                                        bank = psv[ts * 4 + tt][:]
                                        nc.tensor.matmul(
                                            bank, lhsT=shi[:, sl2, tsl], rhs=vwh[:],
                                            start=(dp == 0), stop=False, perf_mode=DRM,
                                        )
                                        nc.tensor.matmul(
                                            bank, lhsT=slo[:, sl2, tsl], rhs=vwh[:],
                                            start=False, stop=False, perf_mode=DRM,
                                        )
                                        nc.tensor.matmul(
                                            bank, lhsT=shi[:, sl2, tsl], rhs=vwl[:],
                                            start=False, stop=(dp == NDP - 1),
                                            perf_mode=DRM,
                                        )
                            for ts in range(2):
                                for tt in range(4):
                                    kb = (2 * pr + ts) * 4 + tt
                                    vsb = evpool.tile([P, 512], FP16, tag="evb", name="vsb")
                                    nc.vector.scalar_tensor_tensor(
                                        vsb[:], psv[ts * 4 + tt][:], 1.0 / WSC,
                                        vb_sb[:, hs * 512 : (hs + 1) * 512], MULT, ADD,
                                    )
                                    nc.sync.dma_start(out=v_i[hs, kb], in_=vsb[:])

            p01_cm.__exit__(None, None, None)
            kv_cm = tc.tile_pool(name="kvS", bufs=4)
            kvpool = kv_cm.__enter__()
            qt_cm = tc.tile_pool(name="qtS", bufs=3)
            qtpool = qt_cm.__enter__()
            pt_cm = tc.tile_pool(name="ptS", bufs=6)
            ptpool = pt_cm.__enter__()
            r_cm = tc.tile_pool(name="rS", bufs=2)
            rpool = r_cm.__enter__()
            p0s_cm = tc.tile_pool(name="p0strip", bufs=4)
            p0strip_pool = p0s_cm.__enter__()

            # ============ P0: Q projection + rotary -> qT_i ============
            p0c_cm = tc.tile_pool(name="p0c", bufs=1)
            p0c = p0c_cm.__enter__()
            cos_own_sb = p0c.tile([64, 2, CH], F32, tag="coso")
            sin_own_sb = p0c.tile([64, 2, CH], F32, tag="sino")
            nc.sync.dma_start(out=cos_own_sb[:], in_=cos_own[:])
            nc.sync.dma_start(out=sin_own_sb[:], in_=sin_own[:])
            qbT_e = p0c.tile([64, QH], F32, tag="qbte")
            qbT_o = p0c.tile([64, QH], F32, tag="qbto")
            nc.sync.dma_start(out=qbT_e[:], in_=qbT[0:64, :])
            nc.sync.dma_start(out=qbT_o[:], in_=qbT[64:P, :])

            def p0_heads(sl, yield_every):
                """Emit Q-proj for one slot (3-pass DR); yields as feeder."""
                shi = p0strip_pool.tile([P, ND, CH], FP8, tag="p0strip", name=f"oh{sl}")
                nc.sync.dma_start(out=shi[:], in_=own_hi[sl])
                slo = p0strip_pool.tile([P, ND, CH], FP8, tag="p0strip", name=f"ol{sl}")
                nc.sync.dma_start(out=slo[:], in_=own_lo[sl])
                for h in range(QH):
                    whi = wpool.tile([P, ND, P], FP8, tag="w", name=f"qwh{sl}_{h}")
                    nc.gpsimd.dma_start(out=whi[:], in_=qw_hi[h])
                    wlo = wpool.tile([P, ND, P], FP8, tag="wl", name=f"qwl{sl}_{h}")
                    nc.gpsimd.dma_start(out=wlo[:], in_=qw_lo[h])
                    ps = pspool.tile([P, CH], F32, tag="ps", name="ps_q")
                    yield from proj3(ps, whi, wlo, shi, slo, yield_every=yield_every)
                    qrot = evpool.tile([P, CH], FP16, tag="ev", name="qrot")
                    rotary_evict(
                        ps, qrot,
                        cos_own_sb[:, sl, :], sin_own_sb[:, sl, :],
                        qbT_e[:, h : h + 1], qbT_o[:, h : h + 1],
                    )
                    nc.sync.dma_start(out=qT_i[sl, h], in_=qrot[:])

            for _ in p0_heads(0, 0):
                pass

            def p4_half(hf, otr8, otrb, wb4pool, ob_sb):
                """O proj for token-slot half hf: 1-pass fp8 DR; slot-0 token
                cols 0-127 (tsub 0) stay bf16.  Yields once per matmul."""
                for hq in range(0, QH, 8):
                    nc.sync.dma_start(
                        out=otr8[:, hq : hq + 8, :],
                        in_=oT8_i[hf, hq : hq + 8].rearrange("h p t -> p h t"),
                    )
                    if hf == 0:
                        nc.sync.dma_start(
                            out=otrb[:, hq : hq + 8, :],
                            in_=oT_lo_i[hq : hq + 8].rearrange("h p t -> p h t"),
                        )
                for e in range(8):
                    ps4 = [
                        pspool.tile([P, 512], F32, tag="ps", name=f"ps4_{i}")
                        for i in range(4)
                    ]
                    for f4 in range(ND // 4):
                        ow8 = wb4pool.tile([P, 4, 512], FP8, tag="wb4", name="ow8")
                        nc.gpsimd.dma_start(
                            out=ow8[:],
                            in_=owT_8[e, 4 * f4 : 4 * f4 + 4].rearrange("d p j -> p d j"),
                        )
                        if hf == 0:
                            owl = wb4pool.tile([P, 4, 512], FP8, tag="wbb", name="owl")
                            nc.gpsimd.dma_start(
                                out=owl[:],
                                in_=owT_lo[e, 4 * f4 : 4 * f4 + 4].rearrange(
                                    "d p j -> p d j"
                                ),
                            )
                        for dfp in range(2):
                            ft = 4 * f4 + 2 * dfp
                            wsl = slice(2 * dfp, 2 * dfp + 2)
                            if hf == 0:
                                nc.tensor.matmul(
                                    ps4[0][:], lhsT=otr8[:, ft : ft + 2, 0:P],
                                    rhs=ow8[:, wsl, :],
                                    start=(ft == 0), stop=False, perf_mode=DRM,
                                )
                                yield
                                nc.tensor.matmul(
                                    ps4[0][:], lhsT=otrb[:, ft : ft + 2, :],
                                    rhs=ow8[:, wsl, :],
                                    start=False, stop=False, perf_mode=DRM,
                                )
                                yield
                                nc.tensor.matmul(
                                    ps4[0][:], lhsT=otr8[:, ft : ft + 2, 0:P],
                                    rhs=owl[:, wsl, :],
                                    start=False, stop=(ft == ND - 2),
                                    perf_mode=DRM,
                                )
                                yield
                            else:
                                nc.tensor.matmul(
                                    ps4[0][:], lhsT=otr8[:, ft : ft + 2, 0:P],
                                    rhs=ow8[:, wsl, :],
                                    start=(ft == 0), stop=(ft == ND - 2),
                                    perf_mode=DRM,
                                )
                                yield
                            for tsub in range(1, 4):
                                nc.tensor.matmul(
                                    ps4[tsub][:],
                                    lhsT=otr8[:, ft : ft + 2,
                                             tsub * P : (tsub + 1) * P],
                                    rhs=ow8[:, wsl, :],
                                    start=(ft == 0), stop=(ft == ND - 2),
                                    perf_mode=DRM,
                                )
                                yield
                    for tsub in range(4):
                        osb = evpool.tile([P, 512], F32, tag="ev4", name="osb4")
                        obias = ob_sb[:, e * 512 : (e + 1) * 512]
                        nc.vector.scalar_tensor_tensor(
                            osb[:], ps4[tsub][:], 1.0 / (OSC * WSC),
                            obias, MULT, ADD,
                        )
                        nc.sync.dma_start(
                            out=out[hf * 4 + tsub, :, e * 512 : (e + 1) * 512],
                            in_=osb[:],
                        )

            def attn_slot(sl, feeder):
                n_kb = NKB[sl]
                with (
                    tc.tile_pool(name=f"mask{sl}", bufs=1) as mpool,
                    tc.tile_pool(name=f"v4{sl}", bufs=1) as v4pool,
                ):
                    msk = mpool.tile([P, 8, CH], FP16, tag="mask", name="msk")
                    nc.sync.dma_start(
                        out=msk[:], in_=masks[sl].rearrange("m k q -> k m q")
                    )
                    for hs in range(2):
                        v4 = v4pool.tile([P, n_kb, 512], FP16, tag="v4", name="v4")
                        nc.sync.dma_start(
                            out=v4[:, 0:n_kb, :],
                            in_=v_i[hs, 0:n_kb].rearrange("b p j -> p b j"),
                        )
                        for j in range(4):
                            kv = 4 * hs + j
                            kt = kvpool.tile([P, n_kb * P], FP16, tag="kt", name="kt")
                            nc.sync.dma_start(
                                out=kt[:, 0 : n_kb * P], in_=kT_i[kv, :, 0 : n_kb * P]
                            )
                            qt4 = qtpool.tile([P, 4, CH], FP16, tag="qt", name="qt4")
                            nc.sync.dma_start(
                                out=qt4[:],
                                in_=qT_i[sl, kv :: KVH].rearrange("g p t -> p g t"),
                            )
                            for g in range(4):
                                h = kv + KVH * g
                                oT_ps = pspool.tile([P, CH], F32, tag="ps", name="oT_ps")
                                sums_ps = pspool.tile([P, CH], F32, tag="ps", name="sums_ps")
                                for kb in range(n_kb):
                                    st_ps = pspool.tile([P, CH], F32, tag="ps", name="st_ps")
                                    nc.tensor.matmul(
                                        st_ps[:],
                                        lhsT=kt[:, kb * P : (kb + 1) * P],
                                        rhs=qt4[:, g, :], start=True, stop=True,
                                    )
                                    pt = ptpool.tile([P, CH], FP16, tag="pt", name="pt")
                                    nc.scalar.activation(
                                        pt[:], st_ps[:], EXP,
                                        scale=SCALE, bias=expb_sb[:],
                                    )
                                    if sl == 0 or kb >= 8:
                                        mi = kb if sl == 0 else kb - 8
                                        nc.vector.tensor_mul(pt[:], pt[:], msk[:, mi, :])
                                    if feeder is not None:
                                        next(feeder, None)
                                    if kb % 2 == 0:
                                        pt_prev = pt
                                    else:
                                        pp = ptpool.tile([P, CH], FP16, tag="ptp", name="pp")
                                        nc.vector.tensor_add(pp[:], pt_prev[:], pt[:])
                                        if kb % 4 == 1:
                                            pp_prev = pp
                                        else:
                                            pq = ptpool.tile([P, CH], FP16, tag="ptq", name="pq")
                                            nc.vector.tensor_add(pq[:], pp_prev[:], pp[:])
                                            nc.tensor.matmul(
                                                sums_ps[0:1, :], lhsT=ones_col[:], rhs=pq[:],
                                                start=(kb == 3), stop=(kb == n_kb - 1),
                                            )
                                    if feeder is not None:
                                        next(feeder, None)
                                    nc.tensor.matmul(
                                        oT_ps[:],
                                        lhsT=v4[:, kb, j * P : (j + 1) * P],
                                        rhs=pt[:],
                                        start=(kb == 0), stop=(kb == n_kb - 1),
                                    )
                                rsb = rpool.tile([1, CH], F32R, tag="r", name="rsb")
                                with nc.allow_low_precision(reason="f32r softmax denom"):
                                    nc.vector.reciprocal(rsb[:], sums_ps[0:1, :])
                                rb_bc = ptpool.tile([P, CH], F32R, tag="ptr", name="rb_bc")
                                nc.gpsimd.partition_broadcast(rb_bc[:], rsb[:])
                                osb8 = evpool.tile([P, CH], FP8, tag="ev8", name="osb8")
                                nc.vector.scalar_tensor_tensor(
                                    osb8[:], oT_ps[:], OSC, rb_bc[:], MULT, MULT
                                )
                                nc.sync.dma_start(out=oT8_i[sl, h], in_=osb8[:])
                                if sl == 0:
                                    o16 = evpool.tile([P, P], F32, tag="evb", name="o16")
                                    nc.vector.scalar_tensor_tensor(
                                        o16[:], oT_ps[:, 0:P], OSC, rb_bc[:, 0:P],
                                        MULT, MULT,
                                    )
                                    olo8 = evpool.tile([P, P], FP8, tag="evl", name="olo8")
                                    nc.vector.tensor_sub(olo8[:], o16[:], osb8[:, 0:P])
                                    nc.sync.dma_start(out=oT_lo_i[h], in_=olo8[:])
                    if feeder is not None:
                        for _ in feeder:
                            pass

            # ==== P3 slot 0 woven with P0 slot 1 ====
            attn_slot(0, p0_heads(1, 3))
            p0c_cm.__exit__(None, None, None)
            p0s_cm.__exit__(None, None, None)

            # ==== P3 slot 1 woven with P4 half 0; then P4 half 1 ====
            with tc.tile_pool(name="obp", bufs=1) as obp:
                ob_sb = obp.tile([P, D], F32, tag="ob")
                nc.sync.dma_start(
                    out=ob_sb[:], in_=ob.ap()[None, :].partition_broadcast(P)
                )
                with (
                    tc.tile_pool(name="p4a", bufs=1) as p4a,
                    tc.tile_pool(name="wb4a", bufs=4) as wb4a,
                ):
                    otr8a = p4a.tile([P, QH, CH], FP8, tag="ot8a")
                    otrba = p4a.tile([P, QH, P], FP8, tag="otba")
                    attn_slot(1, p4_half(0, otr8a, otrba, wb4a, ob_sb))
                with (
                    tc.tile_pool(name="p4b", bufs=1) as p4b,
                    tc.tile_pool(name="wb4b", bufs=6) as wb4b,
                ):
                    otr8b = p4b.tile([P, QH, CH], FP8, tag="ot8b")
                    for _ in p4_half(1, otr8b, None, wb4b, ob_sb):
                        pass
            r_cm.__exit__(None, None, None)
            pt_cm.__exit__(None, None, None)
            qt_cm.__exit__(None, None, None)
            kv_cm.__exit__(None, None, None)
            w_cm.__exit__(None, None, None)

    nc.compile()
    return nc


def _get_nc():
    if "nc" not in _CACHE:
        _CACHE["nc"] = _build()
    return _CACHE["nc"]


_PERM = np.concatenate([np.arange(0, P, 2), np.arange(1, P, 2)])


def _hi_lo(a):
    """fp8 residual split: hi = e4m3(a); lo = e4m3(a - hi)."""
    c = np.ascontiguousarray
    hi = a.astype(np.float32).astype(E4)
    lo = (a.astype(np.float32) - hi.astype(np.float32)).astype(E4)
    return c(hi), c(lo)


def _prep_shared(qw_w, qw_b, kw_w, kw_b, vw_w, vw_b, ow_w, ow_b, fc, fs):
    f32 = np.float32
    c = np.ascontiguousarray
    # [h, dp, dt, fp] = w[h*128 + perm[fp], dt*128 + dp]
    qq = qw_w.reshape(QH, P, D)[:, _PERM, :]                      # [h, fp, d]
    qwT = qq.reshape(QH, P, ND, P).transpose(0, 3, 2, 1)
    kk = kw_w.reshape(KVH, P, D)[:, _PERM, :]
    kwT = kk.reshape(KVH, P, ND, P).transpose(0, 3, 2, 1)
    # [hs, dt, dp, j] = vw[hs*512 + j, dt*128 + dp]
    vwT = vw_w.reshape(2, 512, ND, P).transpose(0, 2, 3, 1)
    # [es, ft, fp, j] = ow[es*512 + j, ft*128 + fp]
    owT = ow_w.reshape(8, 512, ND, P).transpose(0, 2, 3, 1)
    ow_hi, ow_lo = _hi_lo(owT * WSC)
    qw_hi, qw_lo = _hi_lo(qwT * WSC)
    kw_hi, kw_lo = _hi_lo(kwT * WSC)
    vw_hi, vw_lo = _hi_lo(vwT * WSC)
    # rotary tables pre-divided by WSC; proj biases pre-multiplied
    cos_all = c(fc.T.astype(f32) / WSC)  # [64, S]
    sin_all = c(fs.T.astype(f32) / WSC)
    qbT = c(qw_b.reshape(QH, P)[:, _PERM].T.astype(f32) * WSC)
    kbT = c(kw_b.reshape(KVH, P)[:, _PERM].T.astype(f32) * WSC)
    return dict(
        qw_hi=qw_hi, qw_lo=qw_lo, kw_hi=kw_hi, kw_lo=kw_lo,
        vw_hi=vw_hi, vw_lo=vw_lo,
        owT_8=ow_hi, owT_lo=ow_lo,
        cos_all=cos_all, sin_all=sin_all, qbT=qbT, kbT=kbT,
        vb=c(vw_b.astype(f32)), ob=c(ow_b.astype(f32)),
    )


def _masks_for(chunks):
    m = np.zeros((2, 8, P, CH), F16)
    kp = np.arange(P)[:, None]
    qi = np.arange(CH)[None, :]
    for sl in range(2):
        q0 = chunks[sl] * CH
        for mi in range(8):
            kb = mi if sl == 0 else mi + 8
            m[sl, mi] = (kb * P + kp <= q0 + qi).astype(F16)
    return m


def _core_chunks(core):
    b, par = core // 2, core % 2
    return b, ((0, 3) if par == 0 else (1, 2))


def _make_in_maps(inputs):
    """inputs: dict with the reference's setup_inputs() keys (numpy)."""
    g = lambda k: np.asarray(inputs[k])
    shared = _prep_shared(
        g("qw_w"), g("qw_b"), g("kw_w"), g("kw_b"), g("vw_w"), g("vw_b"),
        g("ow_w"), g("ow_b"), g("freqs_cos"), g("freqs_sin"),
    )
    input = g("input")
    in_maps = []
    prepped = {}
    for core in range(NCORES):
        b, chunks = _core_chunks(core)
        if b not in prepped:
            x = input[b].astype(np.float32)  # [S, D]
            # [s, dp, dt, t] = x[s*512 + t, dt*128 + dp]
            strips = x.reshape(NCH, CH, ND, P).transpose(0, 3, 2, 1)
            prepped[b] = _hi_lo(strips)
        phi, plo = prepped[b]
        cos_own = np.ascontiguousarray(
            np.stack([shared["cos_all"][:, c * CH : (c + 1) * CH] for c in chunks], 1)
        )
        sin_own = np.ascontiguousarray(
            np.stack([shared["sin_all"][:, c * CH : (c + 1) * CH] for c in chunks], 1)
        )
        m = dict(shared)
        m.update(
            ones=np.ones(P, F16),
            own_hi=np.ascontiguousarray(phi[list(chunks)]),
            own_lo=np.ascontiguousarray(plo[list(chunks)]),
            pref_hi=phi, pref_lo=plo,
            cos_own=cos_own, sin_own=sin_own, masks=_masks_for(chunks),
        )
        in_maps.append(m)
    return in_maps


def kernel(input, freqs_cos, freqs_sin, qw_w, qw_b, kw_w, kw_b, vw_w, vw_b,
           ow_w, ow_b, start_pos):
    in_maps = _make_in_maps(dict(
        input=input, freqs_cos=freqs_cos, freqs_sin=freqs_sin,
        qw_w=qw_w, qw_b=qw_b, kw_w=kw_w, kw_b=kw_b, vw_w=vw_w, vw_b=vw_b,
        ow_w=ow_w, ow_b=ow_b,
    ))
    nc = _get_nc()
    res = run_bass_kernel_spmd(nc, in_maps, list(range(NCORES)))

    out = np.empty((B, S, D), np.float32)
    for core in range(NCORES):
        b, chunks = _core_chunks(core)
        r = res.results[core]["out"].reshape(2, CH, D)
        for sl in range(2):
            c0 = chunks[sl] * CH
            out[b, c0 : c0 + CH, :] = r[sl]
    return out
